# revision 1
# baseline (speedup 1.0000x reference)
"""CountVectorizer-as-embedding-bag Trainium2 kernel (v2: DVE reduction).

Computes out[b, :] = sum_s W[token_ids[b, s], :] + bias  (== counts @ W + b
without materializing the [B, V] counts matrix).

Sharding: data-parallel over batch across 8 NeuronCores (128 rows/core).

v1 scattered gathered rows into batch rows with one-hot PE matmuls; on HW
each tiny matmul instruction cost ~2.2us and 213 of them dominated (465us
of a 513us body). v2 eliminates the PE entirely:

  - W is cast to bf16 on the host (halves gather traffic; quantization
    error ~5e-4 rel, gate is 2e-2). Each core's 128x200 token block is
    sorted per row and bucketed by vocab quarter (int16 index range);
    each (row, quarter) segment is padded to a static cap with index 0.
  - One transposed `dma_gather` per quarter (bf16 rows transpose cleanly:
    gathered row -> SBUF column [d, token]) fetches [128 D, 128*CAP_q].
  - One DVE `tensor_reduce` per quarter sums each row's CAP_q-column
    window -> [128 D, 128 rows] f32 partials; three adds merge them and
    a host-computed `corr` input (bias minus the pad-row contributions)
    fixes up the padding in the same add chain.
  - Output leaves the device as [D, rows]; the host transposes.
"""

import numpy as np
import ml_dtypes

import concourse.bacc as bacc
import concourse.mybir as mybir
import concourse.tile as tile
from concourse.bass_utils import run_bass_kernel_spmd

B, S, V, D = 1024, 200, 100000, 128
N_CORES = 8
P = 128
BP = B // N_CORES        # 128 batch rows per core

QBASE = [0, 32768, 65536, 98304]
QROWS = [32768, 32768, 32768, V - 98304]
# per-(row, quarter) slot capacity; seed-0 maxima are [85, 89, 86, 10]
# -> overflow falls back to numpy
CAPQ = [88, 92, 89, 12]
NIDX = [P * c for c in CAPQ]          # tokens per quarter gather
IDC = 8 * sum(CAPQ)                   # int16 idx columns = sum(NIDX)/16

_CACHE: dict = {}


def _build_nc(reps: int = 1, mode: str = "full"):
    nc = bacc.Bacc(
        "TRN2",
        target_bir_lowering=False,
        debug=False,
        num_devices=N_CORES,
        num_swdge_queues=4,
        dynamic_dma_scratch_size=65536,
    )
    f32 = mybir.dt.float32
    bf16 = mybir.dt.bfloat16
    ids = nc.dram_tensor("ids", [P, IDC], mybir.dt.int16, kind="ExternalInput")
    corr = nc.dram_tensor("corr", [P, P], f32, kind="ExternalInput")
    Wb = nc.dram_tensor("Wb", [V, D], bf16, kind="ExternalInput")
    out_t = nc.dram_tensor("out_t", [P, P], f32, kind="ExternalOutput")

    with tile.TileContext(nc) as tc:
        with (
            tc.tile_pool(name="const", bufs=1) as cpool,
            tc.tile_pool(name="gather", bufs=2) as gpool,
            tc.tile_pool(name="red", bufs=2) as rpool,
        ):
            ids_sb = cpool.tile([P, IDC], mybir.dt.int16)
            corr_sb = cpool.tile([P, P], f32)
            out_sb = cpool.tile([P, P], f32)

            nc.sync.dma_start(out=ids_sb[:], in_=ids[:])
            nc.sync.dma_start(out=corr_sb[:], in_=corr[:])

            if mode == "reduce":
                # DVE-only body: static garbage tiles instead of gathers
                fixed_G = []
                for q in range(4):
                    g = cpool.tile([P, NIDX[q]], bf16, name=f"fg{q}")
                    nc.vector.memset(g[:], 0.0)
                    fixed_G.append(g)

            for _rep in range(reps):
                rq = []
                base16 = 0
                for q in range(4):
                    ncol = NIDX[q] // 16
                    if mode == "reduce":
                        G = fixed_G[q]
                    elif mode in ("nt1", "nt4"):
                        # non-transpose timing probe (wrong results)
                        G = gpool.tile([P, NIDX[q]], bf16, tag=f"G{q}")
                        nc.gpsimd.dma_gather(
                            G[:].rearrange("p (c e) -> p c e", e=D),
                            Wb[QBASE[q] : QBASE[q] + QROWS[q]],
                            ids_sb[:, base16 : base16 + ncol],
                            NIDX[q],
                            NIDX[q],
                            D,
                            single_packet=False,
                            queue_num=0 if mode == "nt1" else q,
                        )
                    else:
                        G = gpool.tile([P, NIDX[q]], bf16, tag=f"G{q}")
                        nc.gpsimd.dma_gather(
                            G[:].rearrange("p (j c) -> p j c", j=1),
                            Wb[QBASE[q] : QBASE[q] + QROWS[q]],
                            ids_sb[:, base16 : base16 + ncol],
                            NIDX[q],
                            NIDX[q],
                            D,
                            transpose=True,
                            single_packet=False,
                            # all gathers share queue 0: concurrent transpose
                            # gathers on different queues corrupt each other
                            # through the shared XBAR (probe2.py).
                            queue_num=0,
                        )
                    if mode in ("gather", "nt1", "nt4"):
                        # force a wait on gather completion, ~no DVE work
                        r = rpool.tile([P, 1], f32, tag=f"r{q}")
                        nc.vector.tensor_reduce(
                            out=r[:],
                            in_=G[:, 0 : CAPQ[q]].rearrange(
                                "p (r c) -> p r c", r=1
                            ),
                            axis=mybir.AxisListType.X,
                            op=mybir.AluOpType.add,
                        )
                    else:
                        r = rpool.tile([P, P], f32, tag=f"r{q}")
                        nc.vector.tensor_reduce(
                            out=r[:],
                            in_=G[:].rearrange("p (r c) -> p r c", c=CAPQ[q]),
                            axis=mybir.AxisListType.X,
                            op=mybir.AluOpType.add,
                        )
                    rq.append(r)
                    base16 += ncol

                if mode in ("gather", "nt1", "nt4"):
                    nc.vector.tensor_copy(out=out_sb[:], in_=corr_sb[:])
                else:
                    t01 = rpool.tile([P, P], f32, tag="t01")
                    t23 = rpool.tile([P, P], f32, tag="t23")
                    nc.vector.tensor_tensor(
                        out=t01[:], in0=rq[0][:], in1=rq[1][:],
                        op=mybir.AluOpType.add,
                    )
                    nc.vector.tensor_tensor(
                        out=t23[:], in0=rq[2][:], in1=rq[3][:],
                        op=mybir.AluOpType.add,
                    )
                    nc.vector.tensor_tensor(
                        out=t01[:], in0=t01[:], in1=t23[:],
                        op=mybir.AluOpType.add,
                    )
                    nc.vector.tensor_tensor(
                        out=out_sb[:], in0=t01[:], in1=corr_sb[:],
                        op=mybir.AluOpType.add,
                    )

            nc.sync.dma_start(out=out_t[:], in_=out_sb[:])

    nc.compile()
    return nc


def _get_nc(reps: int = 1, mode: str = "full"):
    key = ("nc", reps, mode)
    if key not in _CACHE:
        _CACHE[key] = _build_nc(reps, mode)
    return _CACHE[key]


def _core_inputs(shard: np.ndarray, w_q0_bf: np.ndarray, b: np.ndarray):
    """shard: [128, 200] int32 -> (ids [128, IDC] int16, corr [128, 128] f32).

    w_q0_bf: [4, 128] f32 — the four quarter-base W rows after bf16 cast.
    Raises ValueError on capacity overflow (caller falls back to numpy).
    """
    st = np.sort(shard, axis=1)  # per-row ascending: quarters contiguous
    id_blocks = []
    npad = np.empty((BP, 4), dtype=np.float64)
    for q in range(4):
        lo = QBASE[q]
        hi = QBASE[q] + QROWS[q]
        msk = (st >= lo) & (st < hi)
        n = msk.sum(axis=1)
        if n.max() > CAPQ[q]:
            raise ValueError(f"cap overflow: {n.max()} > {CAPQ[q]} (q={q})")
        npad[:, q] = CAPQ[q] - n
        # stable-sort rows so quarter tokens come first (still ascending)
        order = np.argsort(~msk, axis=1, kind="stable")[:, : CAPQ[q]]
        vals = np.take_along_axis(st, order, axis=1)
        keep = np.arange(CAPQ[q])[None, :] < n[:, None]
        rel = np.where(keep, vals - lo, 0).astype(np.int16)  # [BP, CAPQ]
        flat = rel.reshape(-1)                               # row-major
        wrapped = flat.reshape(-1, 16).T                     # [16, NIDX/16]
        id_blocks.append(np.tile(wrapped, (8, 1)))           # [128, NIDX/16]
    ids_in = np.ascontiguousarray(np.concatenate(id_blocks, axis=1))
    corr = (b[:, None] - w_q0_bf.T @ npad.T).astype(np.float32)
    assert ids_in.shape == (P, IDC) and corr.shape == (P, P)
    return ids_in, np.ascontiguousarray(corr)


def _in_maps(token_ids, W, b):
    Wb = W.astype(ml_dtypes.bfloat16)
    w_q0_bf = np.stack(
        [Wb[QBASE[q]].astype(np.float64) for q in range(4)]
    )  # [4, 128]
    in_maps = []
    for c in range(N_CORES):
        ids_in, corr = _core_inputs(
            token_ids[c * BP : (c + 1) * BP], w_q0_bf, b.astype(np.float64)
        )
        in_maps.append({"ids": ids_in, "corr": corr, "Wb": Wb})
    return in_maps


def _kernel_numpy(token_ids, W, b):
    out = np.tile(b.astype(np.float32), (B, 1))
    for i in range(B):
        out[i] += W[token_ids[i]].sum(axis=0)
    return out.astype(np.float32)


def kernel(token_ids, W, b, **kwargs):
    token_ids = np.ascontiguousarray(np.asarray(token_ids, dtype=np.int32))
    W = np.ascontiguousarray(np.asarray(W, dtype=np.float32))
    b = np.ascontiguousarray(np.asarray(b, dtype=np.float32))
    assert token_ids.shape == (B, S) and W.shape == (V, D) and b.shape == (D,)

    try:
        in_maps = _in_maps(token_ids, W, b)
    except ValueError:
        # capacity overflow on unexpected data: slow-but-correct path
        return _kernel_numpy(token_ids, W, b)

    nc = _get_nc()
    res = run_bass_kernel_spmd(nc, in_maps, core_ids=list(range(N_CORES)))
    return np.concatenate(
        [res.results[c]["out_t"].T for c in range(N_CORES)], axis=0
    ).astype(np.float32)



# revision 3
# speedup vs baseline: 4.3528x; 4.3528x over previous
"""CountVectorizer Trainium2 kernel (v3: vocab-sharded counts matmul).

Computes out = counts @ W + b  where counts[b, v] = #{s: token_ids[b, s] == v}.

v2 (embedding-bag dma_gather) was SWDGE descriptor-generation bound:
~7.85 ns/gathered-row on the Q7, x35968 rows/core => ~282 us of serial
GpSimd time (358 us total).  v3 replaces the gather with the dense
formulation from the sharding hint: the vocab is sharded 8 ways; each core
streams its [12500, 128] bf16 W shard and a host-built [12500, 1024] fp8
counts shard (counts are small ints, exact in e4m3) and accumulates
   out_c[d, b] = sum_v W[v, d] * counts[v, b]
on the PE as 98 accumulating matmuls (lhsT = W tile [128v, 128d] bf16
stationary, rhs = counts tile [128v, 1024b] fp8 moving, PSUM f32).
The host sums the 8 per-core partials and adds the bias (all f32), so the
only error source is the bf16 W cast (~1.6e-3 rel, gate 2e-2).

Per-core HBM: 3.2 MB W + 12.8 MB counts = 16 MB (~45 us at 358 GB/s);
PE: 98 tiles x 1024 cols = 100k cycles (~42 us warm).  DMAs are chunked
and the matmuls chase the chunks, so the two overlap; a burst of dummy
matmuls at t=0 warms the PE HAM clock gate (1.2 -> 2.4 GHz) while the
first chunks stream.
"""

import numpy as np
import ml_dtypes

import concourse.bacc as bacc
import concourse.mybir as mybir
import concourse.tile as tile
from concourse.bass_utils import run_bass_kernel_spmd

B, S, V, D = 1024, 200, 100000, 128
N_CORES = 8
P = 128
VS = V // N_CORES            # 12500 vocab rows per core
G = 98                       # 128-row tiles per shard (ceil)
VP = G * P                   # 12544 padded shard rows
CNT_CH = 7                   # g-tiles per counts DMA chunk
W_CH = 25                    # g-tiles per W DMA chunk

_CACHE: dict = {}


def _build_nc():
    nc = bacc.Bacc(
        "TRN2",
        target_bir_lowering=False,
        debug=False,
        num_devices=N_CORES,
    )
    f32 = mybir.dt.float32
    bf16 = mybir.dt.bfloat16
    fp8 = mybir.dt.float8e4

    cnt = nc.dram_tensor("cnt", [P, G * B], fp8, kind="ExternalInput")
    wsh = nc.dram_tensor("wsh", [P, G * D], bf16, kind="ExternalInput")
    out_t = nc.dram_tensor("out_t", [P, B], f32, kind="ExternalOutput")

    with tile.TileContext(nc) as tc:
        with (
            tc.tile_pool(name="const", bufs=1) as cpool,
            tc.tile_pool(name="psum", bufs=1, space="PSUM") as ppool,
        ):
            cnt_sb = cpool.tile([P, G * B], fp8)
            w_sb = cpool.tile([P, G * D], bf16)
            out_sb = cpool.tile([P, B], f32)
            warm_sb = cpool.tile([P, 512], bf16)

            # HAM warm-up: dummy matmul chain on garbage weights while the
            # first DMA chunks stream in (PE cold clock is 1.2 GHz; ~4 us of
            # sustained activity unthrottles it to 2.4 GHz).
            nc.vector.memset(warm_sb[:], 0.0)
            pwarm = ppool.tile([P, 512], f32, tag="pwarm")
            for k in range(14):
                nc.tensor.matmul(
                    pwarm[:],
                    warm_sb[:, 0:128],
                    warm_sb[:],
                    start=(k == 0),
                    stop=(k == 13),
                )

            # chunked input streams (Tile adds the per-chunk deps)
            for k in range(0, G, W_CH):
                hi = min(k + W_CH, G)
                nc.sync.dma_start(
                    out=w_sb[:, k * D : hi * D], in_=wsh[:, k * D : hi * D]
                )
            for k in range(0, G, CNT_CH):
                hi = min(k + CNT_CH, G)
                nc.sync.dma_start(
                    out=cnt_sb[:, k * B : hi * B], in_=cnt[:, k * B : hi * B]
                )

            ps0 = ppool.tile([P, 512], f32, tag="ps0")
            ps1 = ppool.tile([P, 512], f32, tag="ps1")
            for g in range(G):
                w_tile = w_sb[:, g * D : (g + 1) * D]
                nc.tensor.matmul(
                    ps0[:],
                    w_tile,
                    cnt_sb[:, g * B : g * B + 512],
                    start=(g == 0),
                    stop=(g == G - 1),
                )
                nc.tensor.matmul(
                    ps1[:],
                    w_tile,
                    cnt_sb[:, g * B + 512 : (g + 1) * B],
                    start=(g == 0),
                    stop=(g == G - 1),
                )

            nc.vector.tensor_copy(out=out_sb[:, 0:512], in_=ps0[:])
            nc.vector.tensor_copy(out=out_sb[:, 512:B], in_=ps1[:])
            nc.sync.dma_start(out=out_t[:], in_=out_sb[:])

    nc.compile()
    return nc


def _get_nc():
    if "nc" not in _CACHE:
        _CACHE["nc"] = _build_nc()
    return _CACHE["nc"]


def _shard_layout(arr2d, ncols):
    """[VP, ncols] -> [128, G*ncols] partition-major: out[p, g*ncols+j] =
    arr2d[g*128 + p, j]."""
    a = arr2d.reshape(G, P, ncols).transpose(1, 0, 2).reshape(P, G * ncols)
    return np.ascontiguousarray(a)


def _in_maps(token_ids, W, b):
    # per-row histogram, int16 (max multiplicity is tiny)
    counts = np.zeros((B, V), dtype=np.int16)
    rows = np.repeat(np.arange(B, dtype=np.int64), S)
    np.add.at(counts, (rows, token_ids.ravel().astype(np.int64)), 1)
    if counts.max() > 16:
        raise ValueError("count > 16 not exact in fp8 e4m3")

    Wb = W.astype(ml_dtypes.bfloat16)
    in_maps = []
    for c in range(N_CORES):
        lo = c * VS
        csh = np.zeros((VP, B), dtype=ml_dtypes.float8_e4m3)
        csh[:VS] = counts[:, lo : lo + VS].T.astype(ml_dtypes.float8_e4m3)
        wshard = np.zeros((VP, D), dtype=ml_dtypes.bfloat16)
        wshard[:VS] = Wb[lo : lo + VS]
        in_maps.append(
            {"cnt": _shard_layout(csh, B), "wsh": _shard_layout(wshard, D)}
        )
    return in_maps


def _kernel_numpy(token_ids, W, b):
    out = np.tile(b.astype(np.float32), (B, 1))
    for i in range(B):
        out[i] += W[token_ids[i]].sum(axis=0)
    return out.astype(np.float32)


def kernel(token_ids, W, b, **kwargs):
    token_ids = np.ascontiguousarray(np.asarray(token_ids, dtype=np.int32))
    W = np.ascontiguousarray(np.asarray(W, dtype=np.float32))
    b = np.ascontiguousarray(np.asarray(b, dtype=np.float32))
    assert token_ids.shape == (B, S) and W.shape == (V, D) and b.shape == (D,)

    try:
        in_maps = _in_maps(token_ids, W, b)
    except ValueError:
        return _kernel_numpy(token_ids, W, b)

    nc = _get_nc()
    res = run_bass_kernel_spmd(nc, in_maps, core_ids=list(range(N_CORES)))
    acc = np.zeros((P, B), dtype=np.float32)
    for c in range(N_CORES):
        acc += np.asarray(res.results[c]["out_t"], dtype=np.float32)
    return (acc.T + b[None, :]).astype(np.float32)


# revision 6
# speedup vs baseline: 5.5987x; 1.2862x over previous
"""CountVectorizer Trainium2 kernel (v3: vocab-sharded counts matmul).

Computes out = counts @ W + b  where counts[b, v] = #{s: token_ids[b, s] == v}.

v2 (embedding-bag dma_gather) was SWDGE descriptor-generation bound:
~7.85 ns/gathered-row on the Q7, x35968 rows/core => ~282 us of serial
GpSimd time (358 us total).  v3 replaces the gather with the dense
formulation from the sharding hint: the vocab is sharded 8 ways; each core
streams its [12500, 128] bf16 W shard and a host-built [12500, 1024] fp8
counts shard (counts are small ints, exact in e4m3) and accumulates
   out_c[d, b] = sum_v W[v, d] * counts[v, b]
on the PE as 98 accumulating matmuls (lhsT = W tile [128v, 128d] bf16
stationary, rhs = counts tile [128v, 1024b] fp8 moving, PSUM f32).
The host sums the 8 per-core partials and adds the bias (all f32), so the
only error source is the bf16 W cast (~1.6e-3 rel, gate 2e-2).

Per-core HBM: 3.2 MB W + 12.8 MB counts = 16 MB (~45 us at 358 GB/s);
PE: 98 tiles x 1024 cols = 100k cycles (~42 us warm).  DMAs are chunked
and the matmuls chase the chunks, so the two overlap; a burst of dummy
matmuls at t=0 warms the PE HAM clock gate (1.2 -> 2.4 GHz) while the
first chunks stream.
"""

import numpy as np
import ml_dtypes

import concourse.bacc as bacc
import concourse.mybir as mybir
import concourse.tile as tile
from concourse.bass_utils import run_bass_kernel_spmd

B, S, V, D = 1024, 200, 100000, 128
N_CORES = 8
P = 128
VS = V // N_CORES            # 12500 vocab rows per core
G = 98                       # 128-row tiles per shard (ceil)
VP = G * P                   # 12544 padded shard rows
CNT_CH = 7                   # g-tiles per counts DMA chunk
W_CH = 7                     # g-tiles per W DMA chunk

_CACHE: dict = {}


def _build_nc():
    nc = bacc.Bacc(
        "TRN2",
        target_bir_lowering=False,
        debug=False,
        num_devices=N_CORES,
    )
    f32 = mybir.dt.float32
    bf16 = mybir.dt.bfloat16
    fp8 = mybir.dt.float8e4

    cnt = nc.dram_tensor("cnt", [P, G * B], fp8, kind="ExternalInput")
    wsh = nc.dram_tensor("wsh", [P, G * D], bf16, kind="ExternalInput")
    out_t = nc.dram_tensor("out_t", [P, B], f32, kind="ExternalOutput")

    with tile.TileContext(nc) as tc:
        with (
            tc.tile_pool(name="const", bufs=1) as cpool,
            tc.tile_pool(name="psum", bufs=1, space="PSUM") as ppool,
        ):
            cnt_sb = cpool.tile([P, G * B], fp8)
            w_sb = cpool.tile([P, G * D], bf16)
            out_sb = cpool.tile([P, B], f32)
            warm_sb = cpool.tile([P, 512], bf16)

            # HAM warm-up: dummy matmul chain on garbage weights while the
            # first DMA chunks stream in (PE cold clock is 1.2 GHz; ~4 us of
            # sustained activity unthrottles it to 2.4 GHz).
            nc.vector.memset(warm_sb[:], 0.0)
            pwarm = ppool.tile([P, 512], f32, tag="pwarm")
            for k in range(14):
                nc.tensor.matmul(
                    pwarm[:],
                    warm_sb[:, 0:128],
                    warm_sb[:],
                    start=(k == 0),
                    stop=(k == 13),
                )

            # chunked input streams, W/counts pairwise interleaved so the
            # g-th matmul's operands land together (Tile adds per-chunk deps)
            assert G % CNT_CH == 0 and CNT_CH == W_CH
            for k in range(0, G, CNT_CH):
                hi = k + CNT_CH
                nc.sync.dma_start(
                    out=w_sb[:, k * D : hi * D], in_=wsh[:, k * D : hi * D]
                )
                nc.sync.dma_start(
                    out=cnt_sb[:, k * B : hi * B], in_=cnt[:, k * B : hi * B]
                )

            ps0 = ppool.tile([P, 512], f32, tag="ps0")
            ps1 = ppool.tile([P, 512], f32, tag="ps1")
            for g in range(G):
                w_tile = w_sb[:, g * D : (g + 1) * D]
                nc.tensor.matmul(
                    ps0[:],
                    w_tile,
                    cnt_sb[:, g * B : g * B + 512],
                    start=(g == 0),
                    stop=(g == G - 1),
                )
                nc.tensor.matmul(
                    ps1[:],
                    w_tile,
                    cnt_sb[:, g * B + 512 : (g + 1) * B],
                    start=(g == 0),
                    stop=(g == G - 1),
                )

            # drain per half so copy/out overlap the other half's finish
            nc.vector.tensor_copy(out=out_sb[:, 0:512], in_=ps0[:])
            nc.sync.dma_start(out=out_t[:, 0:512], in_=out_sb[:, 0:512])
            nc.vector.tensor_copy(out=out_sb[:, 512:B], in_=ps1[:])
            nc.sync.dma_start(out=out_t[:, 512:B], in_=out_sb[:, 512:B])

    nc.compile()
    return nc


def _get_nc():
    if "nc" not in _CACHE:
        _CACHE["nc"] = _build_nc()
    return _CACHE["nc"]


def _shard_layout(arr2d, ncols):
    """[VP, ncols] -> [128, G*ncols] partition-major: out[p, g*ncols+j] =
    arr2d[g*128 + p, j]."""
    a = arr2d.reshape(G, P, ncols).transpose(1, 0, 2).reshape(P, G * ncols)
    return np.ascontiguousarray(a)


def _in_maps(token_ids, W, b):
    # per-row histogram, int16 (max multiplicity is tiny)
    counts = np.zeros((B, V), dtype=np.int16)
    rows = np.repeat(np.arange(B, dtype=np.int64), S)
    np.add.at(counts, (rows, token_ids.ravel().astype(np.int64)), 1)
    if counts.max() > 16:
        raise ValueError("count > 16 not exact in fp8 e4m3")

    Wb = W.astype(ml_dtypes.bfloat16)
    in_maps = []
    for c in range(N_CORES):
        lo = c * VS
        csh = np.zeros((VP, B), dtype=ml_dtypes.float8_e4m3)
        csh[:VS] = counts[:, lo : lo + VS].T.astype(ml_dtypes.float8_e4m3)
        wshard = np.zeros((VP, D), dtype=ml_dtypes.bfloat16)
        wshard[:VS] = Wb[lo : lo + VS]
        in_maps.append(
            {"cnt": _shard_layout(csh, B), "wsh": _shard_layout(wshard, D)}
        )
    return in_maps


def _kernel_numpy(token_ids, W, b):
    out = np.tile(b.astype(np.float32), (B, 1))
    for i in range(B):
        out[i] += W[token_ids[i]].sum(axis=0)
    return out.astype(np.float32)


def kernel(token_ids, W, b, **kwargs):
    token_ids = np.ascontiguousarray(np.asarray(token_ids, dtype=np.int32))
    W = np.ascontiguousarray(np.asarray(W, dtype=np.float32))
    b = np.ascontiguousarray(np.asarray(b, dtype=np.float32))
    assert token_ids.shape == (B, S) and W.shape == (V, D) and b.shape == (D,)

    try:
        in_maps = _in_maps(token_ids, W, b)
    except ValueError:
        return _kernel_numpy(token_ids, W, b)

    nc = _get_nc()
    res = run_bass_kernel_spmd(nc, in_maps, core_ids=list(range(N_CORES)))
    acc = np.zeros((P, B), dtype=np.float32)
    for c in range(N_CORES):
        acc += np.asarray(res.results[c]["out_t"], dtype=np.float32)
    return (acc.T + b[None, :]).astype(np.float32)


# revision 7
# speedup vs baseline: 5.6843x; 1.0153x over previous
"""CountVectorizer Trainium2 kernel (v3: vocab-sharded counts matmul).

Computes out = counts @ W + b  where counts[b, v] = #{s: token_ids[b, s] == v}.

v2 (embedding-bag dma_gather) was SWDGE descriptor-generation bound:
~7.85 ns/gathered-row on the Q7, x35968 rows/core => ~282 us of serial
GpSimd time (358 us total).  v3 replaces the gather with the dense
formulation from the sharding hint: the vocab is sharded 8 ways; each core
streams its [12500, 128] bf16 W shard and a host-built [12500, 1024] fp8
counts shard (counts are small ints, exact in e4m3) and accumulates
   out_c[d, b] = sum_v W[v, d] * counts[v, b]
on the PE as 98 accumulating matmuls (lhsT = W tile [128v, 128d] bf16
stationary, rhs = counts tile [128v, 1024b] fp8 moving, PSUM f32).
The host sums the 8 per-core partials and adds the bias (all f32), so the
only error source is the bf16 W cast (~1.6e-3 rel, gate 2e-2).

Per-core HBM: 3.2 MB W + 12.8 MB counts = 16 MB (~45 us at 358 GB/s);
PE: 98 tiles x 1024 cols = 100k cycles (~42 us warm).  DMAs are chunked
and the matmuls chase the chunks, so the two overlap; a burst of dummy
matmuls at t=0 warms the PE HAM clock gate (1.2 -> 2.4 GHz) while the
first chunks stream.
"""

import numpy as np
import ml_dtypes

import concourse.bacc as bacc
import concourse.mybir as mybir
import concourse.tile as tile
from concourse.bass_utils import run_bass_kernel_spmd

B, S, V, D = 1024, 200, 100000, 128
N_CORES = 8
P = 128
VS = V // N_CORES            # 12500 vocab rows per core
G = 98                       # 128-row tiles per shard (ceil)
VP = G * P                   # 12544 padded shard rows
CNT_CH = 7                   # g-tiles per counts DMA chunk
W_CH = 7                     # g-tiles per W DMA chunk

_CACHE: dict = {}


def _build_nc():
    nc = bacc.Bacc(
        "TRN2",
        target_bir_lowering=False,
        debug=False,
        num_devices=N_CORES,
    )
    f32 = mybir.dt.float32
    bf16 = mybir.dt.bfloat16
    fp8 = mybir.dt.float8e4

    cnt = nc.dram_tensor("cnt", [P, G * B], fp8, kind="ExternalInput")
    wsh = nc.dram_tensor("wsh", [P, G * D], bf16, kind="ExternalInput")
    out_t = nc.dram_tensor("out_t", [P, B], f32, kind="ExternalOutput")

    with tile.TileContext(nc) as tc:
        with (
            tc.tile_pool(name="const", bufs=1) as cpool,
            tc.tile_pool(name="psum", bufs=1, space="PSUM") as ppool,
        ):
            cnt_sb = cpool.tile([P, G * B], fp8)
            w_sb = cpool.tile([P, G * D], bf16)
            out_sb = cpool.tile([P, B], f32)

            # (a HAM warm-up chain was tried here and removed: prepending
            # ~5 us of dummy matmuls costs more than the ~1.7 us the cold
            # 1.2 GHz ramp loses on the first ~8 real matmuls)

            # chunked input streams, W/counts pairwise interleaved so the
            # g-th matmul's operands land together (Tile adds per-chunk deps)
            assert G % CNT_CH == 0 and CNT_CH == W_CH
            for k in range(0, G, CNT_CH):
                hi = k + CNT_CH
                nc.sync.dma_start(
                    out=w_sb[:, k * D : hi * D], in_=wsh[:, k * D : hi * D]
                )
                nc.sync.dma_start(
                    out=cnt_sb[:, k * B : hi * B], in_=cnt[:, k * B : hi * B]
                )

            ps0 = ppool.tile([P, 512], f32, tag="ps0")
            ps1 = ppool.tile([P, 512], f32, tag="ps1")
            for g in range(G):
                w_tile = w_sb[:, g * D : (g + 1) * D]
                nc.tensor.matmul(
                    ps0[:],
                    w_tile,
                    cnt_sb[:, g * B : g * B + 512],
                    start=(g == 0),
                    stop=(g == G - 1),
                )
                nc.tensor.matmul(
                    ps1[:],
                    w_tile,
                    cnt_sb[:, g * B + 512 : (g + 1) * B],
                    start=(g == 0),
                    stop=(g == G - 1),
                )

            # drain per half so copy/out overlap the other half's finish
            nc.vector.tensor_copy(out=out_sb[:, 0:512], in_=ps0[:])
            nc.sync.dma_start(out=out_t[:, 0:512], in_=out_sb[:, 0:512])
            nc.vector.tensor_copy(out=out_sb[:, 512:B], in_=ps1[:])
            nc.sync.dma_start(out=out_t[:, 512:B], in_=out_sb[:, 512:B])

    nc.compile()
    return nc


def _get_nc():
    if "nc" not in _CACHE:
        _CACHE["nc"] = _build_nc()
    return _CACHE["nc"]


def _shard_layout(arr2d, ncols):
    """[VP, ncols] -> [128, G*ncols] partition-major: out[p, g*ncols+j] =
    arr2d[g*128 + p, j]."""
    a = arr2d.reshape(G, P, ncols).transpose(1, 0, 2).reshape(P, G * ncols)
    return np.ascontiguousarray(a)


def _in_maps(token_ids, W, b):
    # per-row histogram, int16 (max multiplicity is tiny)
    counts = np.zeros((B, V), dtype=np.int16)
    rows = np.repeat(np.arange(B, dtype=np.int64), S)
    np.add.at(counts, (rows, token_ids.ravel().astype(np.int64)), 1)
    if counts.max() > 16:
        raise ValueError("count > 16 not exact in fp8 e4m3")

    Wb = W.astype(ml_dtypes.bfloat16)
    in_maps = []
    for c in range(N_CORES):
        lo = c * VS
        csh = np.zeros((VP, B), dtype=ml_dtypes.float8_e4m3)
        csh[:VS] = counts[:, lo : lo + VS].T.astype(ml_dtypes.float8_e4m3)
        wshard = np.zeros((VP, D), dtype=ml_dtypes.bfloat16)
        wshard[:VS] = Wb[lo : lo + VS]
        in_maps.append(
            {"cnt": _shard_layout(csh, B), "wsh": _shard_layout(wshard, D)}
        )
    return in_maps


def _kernel_numpy(token_ids, W, b):
    out = np.tile(b.astype(np.float32), (B, 1))
    for i in range(B):
        out[i] += W[token_ids[i]].sum(axis=0)
    return out.astype(np.float32)


def kernel(token_ids, W, b, **kwargs):
    token_ids = np.ascontiguousarray(np.asarray(token_ids, dtype=np.int32))
    W = np.ascontiguousarray(np.asarray(W, dtype=np.float32))
    b = np.ascontiguousarray(np.asarray(b, dtype=np.float32))
    assert token_ids.shape == (B, S) and W.shape == (V, D) and b.shape == (D,)

    try:
        in_maps = _in_maps(token_ids, W, b)
    except ValueError:
        return _kernel_numpy(token_ids, W, b)

    nc = _get_nc()
    res = run_bass_kernel_spmd(nc, in_maps, core_ids=list(range(N_CORES)))
    acc = np.zeros((P, B), dtype=np.float32)
    for c in range(N_CORES):
        acc += np.asarray(res.results[c]["out_t"], dtype=np.float32)
    return (acc.T + b[None, :]).astype(np.float32)
